# revision 14
# baseline (speedup 1.0000x reference)
"""Trainium2 Bass kernel for nn_Attention_20091857010765.

8 NeuronCores, pure data parallel over batch (B=8 -> 1 batch element per core).

Per-core dataflow (C=128 channels on SBUF partitions, fp8/bf16 compute,
fp32 PSUM). TRN2 via this runtime pays a ~200ns dispatch tax per
instruction and PSUM caps matmuls at 512 fp32 output columns, so the
design minimizes instruction count everywhere:
  - x ships bf16; GPSIMD casts row-chunks to an fp8 copy for the convs.
  - k (dense 3x3/s2), a (fused dw+pw 3x3/s2) and q (depthwise as diagonal
    matmuls) run on PE in fp8 DoubleRow (2 taps/pass: 4 pairs + 1 tap).
    Conv outputs are stored fp8 (softmax downstream absorbs it; host-sim
    verified < 1e-3 effect on final rel err). Matmuls interleave k/a/q
    streams so consecutive MMs hit different PSUM banks.
  - Conv PSUM->SBUF copies are 2-tile [*,1024] single ops.
  - v = (wv @ x) * illu stays all-bf16 (the error-sensitive path); the
    illu multiply is a [C,1024] DVE tensor_tensor straight from PSUM.
  - fp8 storage halves the transposes, and DMA-transpose of 16-bit units
    (2 adjacent spatial fp8s) lands exactly in DoubleRow
    [spatial-pair, ko, channel] layout -> Gram in 16 chunk-pairs.
  - l2 norms: one ACT Square+accum per tensor (3 ops total).
  - W^T = A2^T A1^T wout^T via two matmuls (no PE transpose).
  - out = W^T @ v in 2-tile blocks, copies alternating DVE/ACT, batched
    output DMAs.

The W axis is stored even-cols-first / odd-cols-second ("split" layout) so
stride-2 conv taps read contiguous runs; illu is permuted identically on
the host and the output is unpermuted after download.
"""

import sys

sys.path.insert(0, "/opt/trn_rl_repo")

import numpy as np
import ml_dtypes

BF16 = ml_dtypes.bfloat16
FP8 = ml_dtypes.float8_e4m3
TAPS = [(dy, dx) for dy in range(3) for dx in range(3)]
DX_C0 = {0: 0, 1: 65, 2: 1}
PAIRS = [(0, 1), (3, 4), (6, 7), (2, 5)]
TAP_LEFT = 8

B, C, H, W = 8, 128, 128, 128
HP = H + 2
S_FULL = H * W  # 16384
S_DS = (H // 2) * (W // 2)  # 4096
N_TILE = 512
NEG = -30.0
WLIST = np.r_[1:128:2, 0:127:2]

_CACHE = {}


def _build(reps=1, loop_n=None, abl=()):
    """abl: phases to DISABLE (timing ablations): 'conv', 'copies', 'v',
    'vmult', 'gram', 'soft', 'f', 'xdma', 'illu', 'trans', 'cast'."""
    abl = set(abl)
    import concourse.bass as bass
    import concourse.tile as tile
    import concourse.mybir as mybir
    from concourse import bacc
    from concourse.bass import ts
    from contextlib import ExitStack

    dt = mybir.dt
    F32, BF, F8 = dt.float32, dt.bfloat16, dt.float8e4
    AF = mybir.ActivationFunctionType
    OP = mybir.AluOpType
    DR = mybir.MatmulPerfMode.DoubleRow

    nc = bacc.Bacc("TRN2", target_bir_lowering=False, debug=False, num_devices=8)

    def din(name, shape, dtyp=BF):
        return nc.dram_tensor(name, shape, dtyp, kind="ExternalInput").ap()

    xpad_d = din("xpad", [C, HP * HP])
    illu_d = din("illu", [C, S_FULL])
    wk_dr_d = din("wk_dr", [C, 4 * 2 * 128], F8)
    wk8_d = din("wk8", [C, 128], F8)
    wa_dr_d = din("wa_dr", [C, 4 * 2 * 64], F8)
    wa8_d = din("wa8", [C, 64], F8)
    wq_dr_d = din("wq_dr", [C, 4 * 2 * 128], F8)
    wq8_d = din("wq8", [C, 128], F8)
    wvT_d = din("wvT", [C, 128])
    woutT_d = din("woutT", [C, 128])
    ta_d = din("ta", [C, 1], F32)
    tv_d = din("tv", [64, 1], F32)
    mask1_d = din("mask1", [C, 64], F32)
    mask2_d = din("mask2", [64, 128], F32)
    out_d = nc.dram_tensor("out", [C, S_FULL], BF, kind="ExternalOutput").ap()

    with tile.TileContext(nc) as tc, ExitStack() as ctx:
        const = ctx.enter_context(tc.tile_pool(name="const", bufs=1))
        big = ctx.enter_context(tc.tile_pool(name="big", bufs=1))
        small = ctx.enter_context(tc.tile_pool(name="small", bufs=2))
        ps_mm = ctx.enter_context(tc.tile_pool(name="psmm", bufs=2, space="PSUM"))
        ps_v = ctx.enter_context(tc.tile_pool(name="psv", bufs=1, space="PSUM"))
        ps_g = ctx.enter_context(tc.tile_pool(name="psg", bufs=1, space="PSUM"))

        def load_const(name, ap_d, shape, dtyp):
            t = const.tile(shape, dtyp, tag=name, name=f"c_{name}")
            nc.sync.dma_start(t[:], ap_d)
            return t

        wk_dr = load_const("wk_dr", wk_dr_d, [C, 4 * 2 * 128], F8)
        wk8 = load_const("wk8", wk8_d, [C, 128], F8)
        wa_dr = load_const("wa_dr", wa_dr_d, [C, 4 * 2 * 64], F8)
        wa8 = load_const("wa8", wa8_d, [C, 64], F8)
        wq_dr = load_const("wq_dr", wq_dr_d, [C, 4 * 2 * 128], F8)
        wq8 = load_const("wq8", wq8_d, [C, 128], F8)
        wvT = load_const("wvT", wvT_d, [C, 128], BF)
        woutT = load_const("woutT", woutT_d, [C, 128], BF)
        ta = load_const("ta", ta_d, [C, 1], F32)
        tv = load_const("tv", tv_d, [64, 1], F32)
        mask1 = load_const("mask1", mask1_d, [C, 64], F32)
        mask2 = load_const("mask2", mask2_d, [64, 128], F32)

        wk_dr4 = wk_dr[:].rearrange("p (k j m) -> p k j m", k=4, j=2)
        wa_dr4 = wa_dr[:].rearrange("p (k j m) -> p k j m", k=4, j=2)
        wq_dr4 = wq_dr[:].rearrange("p (k j m) -> p k j m", k=4, j=2)

        import contextlib
        if loop_n is not None:
            rep_ctx = lambda: tc.For_i(0, loop_n, 1)
        else:
            rep_ctx = contextlib.nullcontext
        for _rep in range(reps):
          with rep_ctx():
            # ---- input loads: x chunks; illu interleaved behind x head ----
            xpad = big.tile([C, HP * HP], BF, tag="xpad")
            xpad8 = big.tile([C, HP * HP], F8, tag="xpad8")
            illu_q = [big.tile([C, 4096], BF, tag=f"illu{g}", name=f"illu{g}")
                      for g in range(4)]
            from concourse.tile_rust import add_dep_helper
            x_dmas = []
            if "xdma" not in abl:
                for j in range(9):
                    lo = 16 * j * HP
                    hi = min(16 * (j + 1), HP) * HP
                    eng = nc.sync if j % 2 == 0 else nc.scalar
                    d = eng.dma_start(xpad[:, lo:hi], xpad_d[:, lo:hi])
                    x_dmas.append(d)
                    if "cast" not in abl:
                        nc.gpsimd.tensor_copy(xpad8[:, lo:hi], xpad[:, lo:hi])
            if "illu" not in abl:
                for g in range(8):
                    eng = nc.scalar if g % 2 == 0 else nc.sync
                    d = eng.dma_start(
                        illu_q[g // 2][:, ts(g % 2, 2048)], illu_d[:, ts(g, 2048)]
                    )
                    if x_dmas:
                        add_dep_helper(d.ins, x_dmas[min(6 + g // 3, 8)].ins,
                                       reason="illu after x head chunks")

            xf = xpad8[:]
            xp3 = xf.rearrange("p (h w) -> p h w", h=HP, w=HP)
            xb3 = xpad[:].rearrange("p (h w) -> p h w", h=HP, w=HP)

            def pair_view(k, r0, nrows=8):
                ta_, tb = PAIRS[k]
                dya, dxa = TAPS[ta_]
                base = (2 * r0 + dya) * HP + DX_C0[dxa]
                sl = xf[:, base : base + nrows * 2 * HP]
                if k < 3:  # delta 65
                    v4 = sl.rearrange("p (r a c) -> p a r c", r=nrows, a=4, c=65)
                    return v4[:, 0:2, :, 0:64]
                else:  # pair (2,5), delta 130
                    v4 = sl.rearrange("p (r j c) -> p j r c", r=nrows, j=2, c=130)
                    return v4[:, :, :, 0:64]

            def tap_view(t, r0, nrows=8):
                dy, dx = TAPS[t]
                c0 = DX_C0[dx]
                return xp3[:, 2 * r0 + dy : 2 * r0 + dy + 2 * nrows : 2,
                           c0 : c0 + 64]

            def xv_view(i):
                return xb3[:, 4 * i + 1 : 4 * i + 5, 1 : 1 + 128]

            # ---- sbuf homes ----
            q_sb = big.tile([C, S_DS], F8, tag="q")
            k_sb = big.tile([C, S_DS], F8, tag="k")
            a_sb = big.tile([64, S_DS], F8, tag="a")
            # packed transposed tensors: 16-bit units = 2 adjacent spatial fp8
            qTp = big.tile([C, 16 * 128], BF, tag="qTp")
            kTp = big.tile([C, 16 * 128], BF, tag="kTp")
            aTp = big.tile([C, 16 * 64], BF, tag="aTp")
            qT3 = qTp[:].rearrange("p (c j) -> p c j", c=16)
            kT3 = kTp[:].rearrange("p (c j) -> p c j", c=16)
            aT3 = aTp[:].rearrange("p (c j) -> p c j", c=16)
            v_sb = big.tile([C, S_FULL], BF, tag="v")
            scr = big.tile([C, S_DS], F8, tag="scr")

            nq2 = small.tile([C, 1], F32, tag="nq2")
            nk2 = small.tile([C, 1], F32, tag="nk2")
            na2 = small.tile([64, 1], F32, tag="na2")

            g1 = ps_g.tile([C, 64], F32, tag="g1")
            g2 = ps_g.tile([64, 128], F32, tag="g2")

            def conv_block(jj, w_dr4, w8, rows, dst, nm):
                """Two conv tiles (jj, jj+1) of one conv in a [*,1024]
                psum tile + one big copy. Tiles rotate k->a->q on 2 bufs;
                the third block's WAR wait is covered by the middle block's
                matmul time."""
                ps = ps_mm.tile([C, 1024], F32, tag="mm", name=f"cv{nm}{jj}")
                if "conv" not in abl:
                    for t in range(2):
                        for k in range(4):
                            nc.tensor.matmul(
                                ps[:rows, ts(t, 512)], w_dr4[:, k],
                                pair_view(k, 8 * (jj + t)),
                                start=(k == 0), stop=False, perf_mode=DR,
                                skip_group_check=True)
                        nc.tensor.matmul(
                            ps[:rows, ts(t, 512)], w8[:],
                            tap_view(TAP_LEFT, 8 * (jj + t)),
                            start=False, stop=True, skip_group_check=True)
                if "copies" not in abl and "conv" not in abl:
                    o = jj * 512
                    nc.scalar.copy(dst[:rows, o : o + 1024], ps[:rows, :])

            def conv_group(jj):
                conv_block(jj, wk_dr4, wk8, 128, k_sb, "k")
                conv_block(jj, wa_dr4, wa8, 64, a_sb, "a")
                conv_block(jj, wq_dr4, wq8, 128, q_sb, "q")

            def v_block(m):
                if "v" in abl:
                    return
                vp = ps_v.tile([C, 1024], F32, tag="vmm", name=f"vp{m}", bufs=1)
                for t in range(2):
                    nc.tensor.matmul(vp[:, ts(t, 512)], wvT[:],
                                     xv_view(2 * m + t),
                                     start=True, stop=True, skip_group_check=True)
                if "vmult" not in abl:
                    nc.vector.tensor_tensor(
                        v_sb[:, ts(m, 1024)], vp[:],
                        illu_q[m // 4][:, ts(m % 4, 1024)], op=OP.mult,
                    )
                else:
                    nc.scalar.copy(v_sb[:, ts(m, 1024)], vp[:])

            def gram_group(h):
                # chunk-pairs 4h..4h+3 on packed fp8 transposes. LDWEIGHTS
                # rejects byte-interleaved dual-fp8 weights, so run plain
                # fp8 MMs on the even/odd strided sub-elements instead.
                if "gram" in abl:
                    return
                for c in range(4 * h, 4 * h + 4):
                    kT8 = kT3[:, c, :].bitcast(F8).rearrange(
                        "p (c j) -> p j c", c=128, j=2)
                    aT8 = aT3[:, c, :].bitcast(F8).rearrange(
                        "p (d j) -> p j d", d=64, j=2)
                    qT8 = qT3[:, c, :].bitcast(F8).rearrange(
                        "p (c j) -> p j c", c=128, j=2)
                    for j in range(2):
                        last = c == 15 and j == 1
                        nc.tensor.matmul(
                            g2[:], aT8[:, j, :], kT8[:, j, :],
                            start=(c == 0 and j == 0), stop=last,
                            skip_group_check=True,
                        )
                        nc.tensor.matmul(
                            g1[:], qT8[:, j, :], aT8[:, j, :],
                            start=(c == 0 and j == 0), stop=last,
                            skip_group_check=True,
                        )

            for g in range(4):  # conv groups of 2 tiles
                conv_group(2 * g)
                for m in range(4 * g, 4 * g + 4):
                    v_block(m)
                if "trans" not in abl and "conv" not in abl:
                    kb = k_sb[:].bitcast(BF)
                    ab = a_sb[:].bitcast(BF)
                    qb = q_sb[:].bitcast(BF)
                    nc.sync.dma_start(kT3[:, 4 * g : 4 * g + 4, :],
                                      kb[:, ts(g, 512)], transpose=True)
                    nc.sync.dma_start(aT3[:, 4 * g : 4 * g + 4, :],
                                      ab[:, ts(g, 512)], transpose=True)
                    nc.sync.dma_start(qT3[:, 4 * g : 4 * g + 4, :],
                                      qb[:, ts(g, 512)], transpose=True)
                if g >= 1:
                    gram_group(g - 1)
                if g == 3 and "copies" not in abl and "conv" not in abl:
                    # consolidated norms: one Square+accum per tensor
                    nc.scalar.activation(scr[:], k_sb[:], AF.Square,
                                         accum_out=nk2[:])
                    nc.scalar.activation(scr[:64, :], a_sb[:], AF.Square,
                                         accum_out=na2[:])
                    nc.scalar.activation(scr[:], q_sb[:], AF.Square,
                                         accum_out=nq2[:])
            gram_group(3)

            if "soft" not in abl:
                # ---- norms -> scales ----
                rq = small.tile([C, 1], F32, tag="rq")
                rk = small.tile([C, 1], F32, tag="rk")
                ra = small.tile([64, 1], F32, tag="ra")
                for n2, r in ((nq2, rq), (nk2, rk), (na2, ra)):
                    tmp = small.tile([n2.shape[0], 1], F32, tag="rtmp",
                                     name="rtmp")
                    nc.vector.reciprocal(tmp[:], n2[:])
                    nc.scalar.activation(r[:], tmp[:], AF.Sqrt)

                scale1 = small.tile([C, 1], F32, tag="scale1")
                scale2 = small.tile([64, 1], F32, tag="scale2")
                nc.vector.tensor_tensor(scale1[:], rq[:], ta[:], op=OP.mult)
                nc.vector.tensor_tensor(scale2[:], ra[:], tv[:], op=OP.mult)

                def sm_tile(rows, cols, nm):
                    t = ps_v.tile([C, 1024], F32, tag="vmm", bufs=1, name=nm)
                    return t[:rows, :cols]

                # column-scale broadcast matrices via ones-matmuls
                raT = small.tile([1, 64], BF, tag="raT")
                rkT = small.tile([1, 128], BF, tag="rkT")
                ones = small.tile([1, 128], BF, tag="ones")
                nc.vector.memset(ones[:], 1.0)
                nc.gpsimd.dma_start(raT[:], ra[:])
                nc.gpsimd.dma_start(rkT[:], rk[:])
                rab_ps = sm_tile(C, 64, "rab_ps")
                rkb_ps = sm_tile(64, 128, "rkb_ps")
                nc.tensor.matmul(rab_ps[:], ones[:], raT[:], start=True, stop=True)
                nc.tensor.matmul(rkb_ps[:], ones[:, :64], rkT[:], start=True,
                                 stop=True)
                rab = small.tile([C, 64], F32, tag="rab")
                rkb = small.tile([64, 128], F32, tag="rkb")
                nc.vector.tensor_copy(rab[:], rab_ps[:])
                nc.vector.tensor_copy(rkb[:], rkb_ps[:])

                def softmax(g_ps, scale_pp, colb, maskb, p_shape, nm):
                    rows = p_shape[0]
                    l = small.tile(p_shape, F32, tag=f"l{nm}", name=f"l{nm}")
                    nc.vector.scalar_tensor_tensor(
                        l[:], g_ps[:], scale_pp[:], colb[:],
                        op0=OP.mult, op1=OP.mult)
                    nc.vector.tensor_tensor(l[:], l[:], maskb[:], op=OP.add)
                    p = small.tile(p_shape, F32, tag=f"p{nm}", name=f"p{nm}")
                    ssum = small.tile([rows, 1], F32, tag=f"ss{nm}",
                                      name=f"ss{nm}")
                    nc.scalar.activation(p[:], l[:], AF.Exp, accum_out=ssum[:])
                    rsum = small.tile([rows, 1], F32, tag=f"rs{nm}",
                                      name=f"rs{nm}")
                    nc.vector.reciprocal(rsum[:], ssum[:])
                    att = small.tile(p_shape, BF, tag=f"att{nm}",
                                     name=f"att{nm}")
                    nc.vector.tensor_scalar(att[:], p[:], rsum[:], None,
                                            op0=OP.mult)
                    return att

                A1 = softmax(g1, scale1, rab, mask1, [C, 64], "1")
                A2 = softmax(g2, scale2, rkb, mask2, [64, 128], "2")

                # ---- Wb = W^T = A2^T A1^T wout^T (two matmuls) ----
                m1_ps = sm_tile(64, 128, "m1_ps")
                nc.tensor.matmul(m1_ps[:], A1[:], woutT[:], start=True, stop=True)
                M1 = small.tile([64, 128], BF, tag="M1")
                nc.vector.tensor_copy(M1[:], m1_ps[:])
                wb_ps = sm_tile(C, 128, "wb_ps")
                nc.tensor.matmul(wb_ps[:], A2[:], M1[:], start=True, stop=True)
                Wb = small.tile([C, 128], BF, tag="Wb")
                nc.vector.tensor_copy(Wb[:], wb_ps[:])
            else:
                Wb = small.tile([C, 128], BF, tag="Wb")

            # ---- final stream ----
            ogs = [big.tile([C, 2048], BF, tag=f"og{g}", name=f"og{g}")
                   for g in range(4)]

            def f_block(m):
                g, mm = m // 2, m % 2
                fp = ps_mm.tile([C, 1024], F32, tag="mm", name=f"fp{m}")
                for t in range(2):
                    nc.tensor.matmul(
                        fp[:, ts(t, 512)], Wb[:],
                        v_sb[:, (2 * m + t) * 512 : (2 * m + t + 1) * 512],
                        start=True, stop=True, skip_group_check=True,
                    )
                if m % 2 == 0:
                    nc.vector.tensor_copy(ogs[g % 4][:, ts(mm, 1024)], fp[:])
                else:
                    nc.scalar.copy(ogs[g % 4][:, ts(mm, 1024)], fp[:])
                if mm == 1:
                    nc.sync.dma_start(out_d[:, ts(g, 2048)], ogs[g % 4][:])

            if "f" not in abl:
                for m in range(16):
                    f_block(m)

    nc.compile()
    return nc


def _split_cols(arr):
    return np.concatenate([arr[..., 0::2], arr[..., 1::2]], axis=-1)


def _prep_inputs(x, illu_feat, wq, wk, wa_dw, wa_pw, wv, wout, temp_a, temp_v):
    xp = np.zeros((B, C, HP, HP), np.float32)
    xp[:, :, 1:-1, 1:-1] = x
    xp = _split_cols(xp).reshape(B, C, HP * HP).astype(BF16)
    il = illu_feat[:, :, :, WLIST].reshape(B, C, S_FULL).astype(BF16)

    wkT = np.empty((C, 9, 128), np.float32)
    waT = np.empty((C, 9, 64), np.float32)
    wqd = np.zeros((C, 9, 128), np.float32)
    for t, (dy, dx) in enumerate(TAPS):
        wkT[:, t, :] = wk[:, :, dy, dx].T
        waT[:, t, :] = (wa_pw[:, :, 0, 0] * wa_dw[None, :, 0, dy, dx]).T
        wqd[np.arange(C), t, np.arange(C)] = wq[:, 0, dy, dx]

    def pack_dr(wT, m):
        dr = np.empty((C, 4, 2, m), np.float32)
        for k, (ta_, tb) in enumerate(PAIRS):
            dr[:, k, 0, :] = wT[:, ta_, :]
            dr[:, k, 1, :] = wT[:, tb, :]
        return dr.reshape(C, 4 * 2 * m).astype(FP8), wT[:, TAP_LEFT, :].astype(FP8)

    wk_dr, wk8 = pack_dr(wkT, 128)
    wa_dr, wa8 = pack_dr(waT, 64)
    wq_dr, wq8 = pack_dr(wqd, 128)

    heads_c = np.arange(C) // 16
    heads_d = np.arange(64) // 8
    mask1 = np.where(heads_d[None, :] == heads_c[:, None], 0.0, NEG).astype(np.float32)
    mask2 = np.where(heads_c[None, :] == heads_d[:, None], 0.0, NEG).astype(np.float32)

    consts = {
        "wk_dr": wk_dr, "wk8": wk8,
        "wa_dr": wa_dr, "wa8": wa8,
        "wq_dr": wq_dr, "wq8": wq8,
        "wvT": np.ascontiguousarray(wv[:, :, 0, 0].T).astype(BF16),
        "woutT": np.ascontiguousarray(wout[:, :, 0, 0].T).astype(BF16),
        "ta": np.repeat(temp_a.ravel(), 16).reshape(C, 1).astype(np.float32),
        "tv": np.repeat(temp_v.ravel(), 8).reshape(64, 1).astype(np.float32),
        "mask1": mask1,
        "mask2": mask2,
    }
    return [dict(consts, xpad=xp[b], illu=il[b]) for b in range(B)]


LAST_RESULTS = None


def kernel(x, illu_feat, wq, wk, wa_dw, wa_pw, wv, wout, temp_a, temp_v):
    global LAST_RESULTS
    reps = int(_CACHE.get("reps", 1))
    loop_n = _CACHE.get("loop_n")
    key = f"nc{reps}_{loop_n}"
    if key not in _CACHE:
        _CACHE[key] = _build(reps, loop_n=loop_n)
    nc = _CACHE[key]

    in_maps = _prep_inputs(
        np.asarray(x, np.float32), np.asarray(illu_feat, np.float32),
        np.asarray(wq, np.float32), np.asarray(wk, np.float32),
        np.asarray(wa_dw, np.float32), np.asarray(wa_pw, np.float32),
        np.asarray(wv, np.float32), np.asarray(wout, np.float32),
        np.asarray(temp_a, np.float32), np.asarray(temp_v, np.float32),
    )

    from concourse.bass_utils import run_bass_kernel_spmd

    res = run_bass_kernel_spmd(nc, in_maps, core_ids=list(range(B)))
    LAST_RESULTS = res
    out = np.stack([np.asarray(res.results[b]["out"], np.float32) for b in range(B)])
    out = out.reshape(B, C, H, W)
    inv = np.empty_like(out)
    inv[:, :, :, WLIST] = out
    return inv


# revision 17
# speedup vs baseline: 1.3338x; 1.3338x over previous
"""Trainium2 Bass kernel for nn_Attention_20091857010765.

8 NeuronCores, pure data parallel over batch (B=8 -> 1 batch element per core).

Per-core dataflow (C=128 channels on SBUF partitions, fp8/bf16 compute,
fp32 PSUM). TRN2 via this runtime pays a ~200ns dispatch tax per
instruction and PSUM caps matmuls at 512 fp32 output columns, so the
design minimizes instruction count everywhere:
  - x ships bf16; GPSIMD casts row-chunks to an fp8 copy for the convs.
  - k (dense 3x3/s2), a (fused dw+pw 3x3/s2) and q (depthwise as diagonal
    matmuls) run on PE in fp8 DoubleRow (2 taps/pass: 4 pairs + 1 tap).
    Conv outputs are stored fp8 (softmax downstream absorbs it; host-sim
    verified < 1e-3 effect on final rel err). Matmuls interleave k/a/q
    streams so consecutive MMs hit different PSUM banks.
  - Conv PSUM->SBUF copies are 2-tile [*,1024] single ops.
  - v = (wv @ x) * illu stays all-bf16 (the error-sensitive path); the
    illu multiply is a [C,1024] DVE tensor_tensor straight from PSUM.
  - fp8 storage halves the transposes, and DMA-transpose of 16-bit units
    (2 adjacent spatial fp8s) lands exactly in DoubleRow
    [spatial-pair, ko, channel] layout -> Gram in 16 chunk-pairs.
  - l2 norms: one ACT Square+accum per tensor (3 ops total).
  - W^T = A2^T A1^T wout^T via two matmuls (no PE transpose).
  - out = W^T @ v in 2-tile blocks, copies alternating DVE/ACT, batched
    output DMAs.

The W axis is stored even-cols-first / odd-cols-second ("split" layout) so
stride-2 conv taps read contiguous runs; illu is permuted identically on
the host and the output is unpermuted after download.
"""

import sys

sys.path.insert(0, "/opt/trn_rl_repo")

import numpy as np
import ml_dtypes

BF16 = ml_dtypes.bfloat16
FP8 = ml_dtypes.float8_e4m3
TAPS = [(dy, dx) for dy in range(3) for dx in range(3)]
DX_C0 = {0: 0, 1: 65, 2: 1}
PAIRS = [(0, 1), (3, 4), (6, 7), (2, 5)]
TAP_LEFT = 8

B, C, H, W = 8, 128, 128, 128
HP = H + 2
S_FULL = H * W  # 16384
S_DS = (H // 2) * (W // 2)  # 4096
N_TILE = 512
NEG = -30.0
WLIST = np.r_[1:128:2, 0:127:2]

_CACHE = {}


def _build(reps=1, loop_n=None, abl=()):
    """abl: phases to DISABLE (timing ablations): 'conv', 'copies', 'v',
    'vmult', 'gram', 'soft', 'f', 'xdma', 'illu', 'trans', 'cast'."""
    abl = set(abl)
    import concourse.bass as bass
    import concourse.tile as tile
    import concourse.mybir as mybir
    from concourse import bacc
    from concourse.bass import ts
    from contextlib import ExitStack

    dt = mybir.dt
    F32, BF, F8 = dt.float32, dt.bfloat16, dt.float8e4
    AF = mybir.ActivationFunctionType
    OP = mybir.AluOpType
    DR = mybir.MatmulPerfMode.DoubleRow

    nc = bacc.Bacc("TRN2", target_bir_lowering=False, debug=False, num_devices=8)

    def din(name, shape, dtyp=BF):
        return nc.dram_tensor(name, shape, dtyp, kind="ExternalInput").ap()

    xpad_d = din("xpad", [C, HP * HP])
    illu_d = din("illu", [C, S_FULL])
    wk_dr_d = din("wk_dr", [C, 4 * 2 * 128], F8)
    wk8_d = din("wk8", [C, 128], F8)
    wa_dr_d = din("wa_dr", [C, 4 * 2 * 64], F8)
    wa8_d = din("wa8", [C, 64], F8)
    wq_dr_d = din("wq_dr", [C, 4 * 2 * 128], F8)
    wq8_d = din("wq8", [C, 128], F8)
    wvT_d = din("wvT", [C, 128])
    woutT_d = din("woutT", [C, 128])
    ta_d = din("ta", [C, 1], F32)
    tv_d = din("tv", [64, 1], F32)
    mask1_d = din("mask1", [C, 64], F32)
    mask2_d = din("mask2", [64, 128], F32)
    out_d = nc.dram_tensor("out", [C, S_FULL], BF, kind="ExternalOutput").ap()

    with tile.TileContext(nc) as tc, ExitStack() as ctx:
        const = ctx.enter_context(tc.tile_pool(name="const", bufs=1))
        big = ctx.enter_context(tc.tile_pool(name="big", bufs=1))
        small = ctx.enter_context(tc.tile_pool(name="small", bufs=2))
        ps_mm = ctx.enter_context(tc.tile_pool(name="psmm", bufs=2, space="PSUM"))
        ps_v = ctx.enter_context(tc.tile_pool(name="psv", bufs=1, space="PSUM"))
        ps_g = ctx.enter_context(tc.tile_pool(name="psg", bufs=1, space="PSUM"))

        def load_const(name, ap_d, shape, dtyp):
            t = const.tile(shape, dtyp, tag=name, name=f"c_{name}")
            nc.sync.dma_start(t[:], ap_d)
            return t

        wk_dr = load_const("wk_dr", wk_dr_d, [C, 4 * 2 * 128], F8)
        wk8 = load_const("wk8", wk8_d, [C, 128], F8)
        wa_dr = load_const("wa_dr", wa_dr_d, [C, 4 * 2 * 64], F8)
        wa8 = load_const("wa8", wa8_d, [C, 64], F8)
        wq_dr = load_const("wq_dr", wq_dr_d, [C, 4 * 2 * 128], F8)
        wq8 = load_const("wq8", wq8_d, [C, 128], F8)
        wvT = load_const("wvT", wvT_d, [C, 128], BF)
        woutT = load_const("woutT", woutT_d, [C, 128], BF)
        ta = load_const("ta", ta_d, [C, 1], F32)
        tv = load_const("tv", tv_d, [64, 1], F32)
        mask1 = load_const("mask1", mask1_d, [C, 64], F32)
        mask2 = load_const("mask2", mask2_d, [64, 128], F32)

        wk_dr4 = wk_dr[:].rearrange("p (k j m) -> p k j m", k=4, j=2)
        wa_dr4 = wa_dr[:].rearrange("p (k j m) -> p k j m", k=4, j=2)
        wq_dr4 = wq_dr[:].rearrange("p (k j m) -> p k j m", k=4, j=2)

        import contextlib
        if loop_n is not None:
            rep_ctx = lambda: tc.For_i(0, loop_n, 1)
        else:
            rep_ctx = contextlib.nullcontext
        for _rep in range(reps):
          with rep_ctx():
            # ---- input loads: x chunks; illu interleaved behind x head ----
            xpad = big.tile([C, HP * HP], BF, tag="xpad")
            xpad8 = big.tile([C, HP * HP], F8, tag="xpad8")
            illu_q = [big.tile([C, 4096], BF, tag=f"illu{g}", name=f"illu{g}")
                      for g in range(4)]
            from concourse.tile_rust import add_dep_helper
            x_dmas = []
            if "xdma" not in abl:
                for j in range(9):
                    lo = 16 * j * HP
                    hi = min(16 * (j + 1), HP) * HP
                    eng = nc.sync if j % 2 == 0 else nc.scalar
                    d = eng.dma_start(xpad[:, lo:hi], xpad_d[:, lo:hi])
                    x_dmas.append(d)
                    if "cast" not in abl:
                        nc.vector.tensor_copy(xpad8[:, lo:hi], xpad[:, lo:hi])
            if "illu" not in abl:
                for g in range(8):
                    eng = nc.scalar if g % 2 == 0 else nc.sync
                    d = eng.dma_start(
                        illu_q[g // 2][:, ts(g % 2, 2048)], illu_d[:, ts(g, 2048)]
                    )
                    if x_dmas:
                        add_dep_helper(d.ins, x_dmas[min(6 + g // 3, 8)].ins,
                                       reason="illu after x head chunks")

            xf = xpad8[:]
            xp3 = xf.rearrange("p (h w) -> p h w", h=HP, w=HP)
            xb3 = xpad[:].rearrange("p (h w) -> p h w", h=HP, w=HP)

            def pair_view(k, r0, nrows=8):
                ta_, tb = PAIRS[k]
                dya, dxa = TAPS[ta_]
                base = (2 * r0 + dya) * HP + DX_C0[dxa]
                sl = xf[:, base : base + nrows * 2 * HP]
                if k < 3:  # delta 65
                    v4 = sl.rearrange("p (r a c) -> p a r c", r=nrows, a=4, c=65)
                    return v4[:, 0:2, :, 0:64]
                else:  # pair (2,5), delta 130
                    v4 = sl.rearrange("p (r j c) -> p j r c", r=nrows, j=2, c=130)
                    return v4[:, :, :, 0:64]

            def tap_view(t, r0, nrows=8):
                dy, dx = TAPS[t]
                c0 = DX_C0[dx]
                return xp3[:, 2 * r0 + dy : 2 * r0 + dy + 2 * nrows : 2,
                           c0 : c0 + 64]

            def xv_view(i):
                return xb3[:, 4 * i + 1 : 4 * i + 5, 1 : 1 + 128]

            # ---- sbuf homes ----
            q_sb = big.tile([C, S_DS], F8, tag="q")
            k_sb = big.tile([C, S_DS], F8, tag="k")
            a_sb = big.tile([64, S_DS], F8, tag="a")
            # packed transposed tensors: 16-bit units = 2 adjacent spatial fp8
            qTp = big.tile([C, 16 * 128], BF, tag="qTp")
            kTp = big.tile([C, 16 * 128], BF, tag="kTp")
            aTp = big.tile([C, 16 * 64], BF, tag="aTp")
            qT3 = qTp[:].rearrange("p (c j) -> p c j", c=16)
            kT3 = kTp[:].rearrange("p (c j) -> p c j", c=16)
            aT3 = aTp[:].rearrange("p (c j) -> p c j", c=16)
            v_sb = big.tile([C, S_FULL], BF, tag="v")
            scr = big.tile([C, 1024], F8, tag="scr")

            nq2p = small.tile([C, 4], F32, tag="nq2p")
            nk2p = small.tile([C, 4], F32, tag="nk2p")
            na2p = small.tile([64, 4], F32, tag="na2p")
            nq2 = small.tile([C, 1], F32, tag="nq2")
            nk2 = small.tile([C, 1], F32, tag="nk2")
            na2 = small.tile([64, 1], F32, tag="na2")

            g1 = ps_g.tile([C, 64], F32, tag="g1")
            g2 = ps_g.tile([64, 128], F32, tag="g2")

            def conv_block(jj, w_dr4, w8, rows, dst, n2p, nm):
                """Two conv tiles (jj, jj+1) of one conv in a [*,1024]
                psum tile + one big copy. Tiles rotate k->a->q on 2 bufs;
                the third block's WAR wait is covered by the middle block's
                matmul time."""
                ps = ps_mm.tile([C, 1024], F32, tag="mm", name=f"cv{nm}{jj}")
                if "conv" not in abl:
                    for t in range(2):
                        for k in range(4):
                            nc.tensor.matmul(
                                ps[:rows, ts(t, 512)], w_dr4[:, k],
                                pair_view(k, 8 * (jj + t)),
                                start=(k == 0), stop=False, perf_mode=DR,
                                skip_group_check=True)
                        nc.tensor.matmul(
                            ps[:rows, ts(t, 512)], w8[:],
                            tap_view(TAP_LEFT, 8 * (jj + t)),
                            start=False, stop=True, skip_group_check=True)
                if "copies" not in abl and "conv" not in abl:
                    o = jj * 512
                    nc.scalar.copy(dst[:rows, o : o + 1024], ps[:rows, :])
                    # norms straight from PSUM: one Square+accum per block
                    nc.scalar.activation(scr[:rows, :], ps[:rows, :],
                                         AF.Square,
                                         accum_out=n2p[:, jj // 2 : jj // 2 + 1])

            def conv_group(jj):
                g = jj // 2
                conv_block(jj, wk_dr4, wk8, 128, k_sb, nk2p, "k")
                v_block(4 * g)
                conv_block(jj, wa_dr4, wa8, 64, a_sb, na2p, "a")
                v_block(4 * g + 1)
                conv_block(jj, wq_dr4, wq8, 128, q_sb, nq2p, "q")
                v_block(4 * g + 2)
                v_block(4 * g + 3)

            def v_block(m):
                if "v" in abl:
                    return
                vp = ps_v.tile([C, 1024], F32, tag="vmm", name=f"vp{m}", bufs=1)
                for t in range(2):
                    nc.tensor.matmul(vp[:, ts(t, 512)], wvT[:],
                                     xv_view(2 * m + t),
                                     start=True, stop=True, skip_group_check=True)
                if "vmult" not in abl:
                    nc.vector.tensor_tensor(
                        v_sb[:, ts(m, 1024)], vp[:],
                        illu_q[m // 4][:, ts(m % 4, 1024)], op=OP.mult,
                    )
                else:
                    nc.scalar.copy(v_sb[:, ts(m, 1024)], vp[:])

            def gram_group(h):
                # chunk-pairs 4h..4h+3 on packed fp8 transposes. LDWEIGHTS
                # rejects byte-interleaved dual-fp8 weights, so run plain
                # fp8 MMs on the even/odd strided sub-elements instead.
                if "gram" in abl:
                    return
                for c in range(4 * h, 4 * h + 4):
                    kT8 = kT3[:, c, :].bitcast(F8).rearrange(
                        "p (c j) -> p j c", c=128, j=2)
                    aT8 = aT3[:, c, :].bitcast(F8).rearrange(
                        "p (d j) -> p j d", d=64, j=2)
                    qT8 = qT3[:, c, :].bitcast(F8).rearrange(
                        "p (c j) -> p j c", c=128, j=2)
                    for j in range(2):
                        last = c == 15 and j == 1
                        nc.tensor.matmul(
                            g2[:], aT8[:, j, :], kT8[:, j, :],
                            start=(c == 0 and j == 0), stop=last,
                            skip_group_check=True,
                        )
                        nc.tensor.matmul(
                            g1[:], qT8[:, j, :], aT8[:, j, :],
                            start=(c == 0 and j == 0), stop=last,
                            skip_group_check=True,
                        )

            for g in range(4):  # conv groups of 2 tiles
                conv_group(2 * g)
                if "trans" not in abl and "conv" not in abl:
                    kb = k_sb[:].bitcast(BF)
                    ab = a_sb[:].bitcast(BF)
                    qb = q_sb[:].bitcast(BF)
                    nc.sync.dma_start(kT3[:, 4 * g : 4 * g + 4, :],
                                      kb[:, ts(g, 512)], transpose=True)
                    nc.sync.dma_start(aT3[:, 4 * g : 4 * g + 4, :],
                                      ab[:, ts(g, 512)], transpose=True)
                    nc.sync.dma_start(qT3[:, 4 * g : 4 * g + 4, :],
                                      qb[:, ts(g, 512)], transpose=True)
                if g >= 1:
                    gram_group(g - 1)
            gram_group(3)
            if "copies" not in abl and "conv" not in abl:
                nc.vector.tensor_reduce(nk2[:], nk2p[:],
                                        axis=mybir.AxisListType.X, op=OP.add)
                nc.vector.tensor_reduce(na2[:], na2p[:],
                                        axis=mybir.AxisListType.X, op=OP.add)
                nc.vector.tensor_reduce(nq2[:], nq2p[:],
                                        axis=mybir.AxisListType.X, op=OP.add)

            if "soft" not in abl:
                # ---- norms -> scales ----
                rq = small.tile([C, 1], F32, tag="rq")
                rk = small.tile([C, 1], F32, tag="rk")
                ra = small.tile([64, 1], F32, tag="ra")
                for n2, r in ((nq2, rq), (nk2, rk), (na2, ra)):
                    tmp = small.tile([n2.shape[0], 1], F32, tag="rtmp",
                                     name="rtmp")
                    nc.vector.reciprocal(tmp[:], n2[:])
                    nc.scalar.activation(r[:], tmp[:], AF.Sqrt)

                scale1 = small.tile([C, 1], F32, tag="scale1")
                scale2 = small.tile([64, 1], F32, tag="scale2")
                nc.vector.tensor_tensor(scale1[:], rq[:], ta[:], op=OP.mult)
                nc.vector.tensor_tensor(scale2[:], ra[:], tv[:], op=OP.mult)

                def sm_tile(rows, cols, nm):
                    t = ps_v.tile([C, 1024], F32, tag="vmm", bufs=1, name=nm)
                    return t[:rows, :cols]

                # column-scale broadcast matrices via ones-matmuls
                raT = small.tile([1, 64], BF, tag="raT")
                rkT = small.tile([1, 128], BF, tag="rkT")
                ones = small.tile([1, 128], BF, tag="ones")
                nc.vector.memset(ones[:], 1.0)
                nc.gpsimd.dma_start(raT[:], ra[:])
                nc.gpsimd.dma_start(rkT[:], rk[:])
                rab_ps = sm_tile(C, 64, "rab_ps")
                rkb_ps = sm_tile(64, 128, "rkb_ps")
                nc.tensor.matmul(rab_ps[:], ones[:], raT[:], start=True, stop=True)
                nc.tensor.matmul(rkb_ps[:], ones[:, :64], rkT[:], start=True,
                                 stop=True)
                rab = small.tile([C, 64], F32, tag="rab")
                rkb = small.tile([64, 128], F32, tag="rkb")
                nc.vector.tensor_copy(rab[:], rab_ps[:])
                nc.vector.tensor_copy(rkb[:], rkb_ps[:])

                def softmax(g_ps, scale_pp, colb, maskb, p_shape, nm):
                    rows = p_shape[0]
                    l = small.tile(p_shape, F32, tag=f"l{nm}", name=f"l{nm}")
                    nc.vector.scalar_tensor_tensor(
                        l[:], g_ps[:], scale_pp[:], colb[:],
                        op0=OP.mult, op1=OP.mult)
                    nc.vector.tensor_tensor(l[:], l[:], maskb[:], op=OP.add)
                    p = small.tile(p_shape, F32, tag=f"p{nm}", name=f"p{nm}")
                    ssum = small.tile([rows, 1], F32, tag=f"ss{nm}",
                                      name=f"ss{nm}")
                    nc.scalar.activation(p[:], l[:], AF.Exp, accum_out=ssum[:])
                    rsum = small.tile([rows, 1], F32, tag=f"rs{nm}",
                                      name=f"rs{nm}")
                    nc.vector.reciprocal(rsum[:], ssum[:])
                    att = small.tile(p_shape, BF, tag=f"att{nm}",
                                     name=f"att{nm}")
                    nc.vector.tensor_scalar(att[:], p[:], rsum[:], None,
                                            op0=OP.mult)
                    return att

                A1 = softmax(g1, scale1, rab, mask1, [C, 64], "1")
                A2 = softmax(g2, scale2, rkb, mask2, [64, 128], "2")

                # ---- Wb = W^T = A2^T A1^T wout^T (two matmuls) ----
                m1_ps = sm_tile(64, 128, "m1_ps")
                nc.tensor.matmul(m1_ps[:], A1[:], woutT[:], start=True, stop=True)
                M1 = small.tile([64, 128], BF, tag="M1")
                nc.vector.tensor_copy(M1[:], m1_ps[:])
                wb_ps = sm_tile(C, 128, "wb_ps")
                nc.tensor.matmul(wb_ps[:], A2[:], M1[:], start=True, stop=True)
                Wb = small.tile([C, 128], BF, tag="Wb")
                nc.vector.tensor_copy(Wb[:], wb_ps[:])
            else:
                Wb = small.tile([C, 128], BF, tag="Wb")

            # ---- final stream ----
            ogs = [big.tile([C, 2048], BF, tag=f"og{g}", name=f"og{g}")
                   for g in range(4)]

            def f_block(m):
                g, mm = m // 2, m % 2
                fp = ps_mm.tile([C, 1024], F32, tag="mm", name=f"fp{m}")
                for t in range(2):
                    nc.tensor.matmul(
                        fp[:, ts(t, 512)], Wb[:],
                        v_sb[:, (2 * m + t) * 512 : (2 * m + t + 1) * 512],
                        start=True, stop=True, skip_group_check=True,
                    )
                if m % 2 == 0:
                    nc.vector.tensor_copy(ogs[g % 4][:, ts(mm, 1024)], fp[:])
                else:
                    nc.scalar.copy(ogs[g % 4][:, ts(mm, 1024)], fp[:])
                if mm == 1:
                    nc.sync.dma_start(out_d[:, ts(g, 2048)], ogs[g % 4][:])

            if "f" not in abl:
                for m in range(16):
                    f_block(m)

    nc.compile()
    return nc


def _split_cols(arr):
    return np.concatenate([arr[..., 0::2], arr[..., 1::2]], axis=-1)


def _prep_inputs(x, illu_feat, wq, wk, wa_dw, wa_pw, wv, wout, temp_a, temp_v):
    xp = np.zeros((B, C, HP, HP), np.float32)
    xp[:, :, 1:-1, 1:-1] = x
    xp = _split_cols(xp).reshape(B, C, HP * HP).astype(BF16)
    il = illu_feat[:, :, :, WLIST].reshape(B, C, S_FULL).astype(BF16)

    wkT = np.empty((C, 9, 128), np.float32)
    waT = np.empty((C, 9, 64), np.float32)
    wqd = np.zeros((C, 9, 128), np.float32)
    for t, (dy, dx) in enumerate(TAPS):
        wkT[:, t, :] = wk[:, :, dy, dx].T
        waT[:, t, :] = (wa_pw[:, :, 0, 0] * wa_dw[None, :, 0, dy, dx]).T
        wqd[np.arange(C), t, np.arange(C)] = wq[:, 0, dy, dx]

    def pack_dr(wT, m):
        dr = np.empty((C, 4, 2, m), np.float32)
        for k, (ta_, tb) in enumerate(PAIRS):
            dr[:, k, 0, :] = wT[:, ta_, :]
            dr[:, k, 1, :] = wT[:, tb, :]
        return dr.reshape(C, 4 * 2 * m).astype(FP8), wT[:, TAP_LEFT, :].astype(FP8)

    wk_dr, wk8 = pack_dr(wkT, 128)
    wa_dr, wa8 = pack_dr(waT, 64)
    wq_dr, wq8 = pack_dr(wqd, 128)

    heads_c = np.arange(C) // 16
    heads_d = np.arange(64) // 8
    mask1 = np.where(heads_d[None, :] == heads_c[:, None], 0.0, NEG).astype(np.float32)
    mask2 = np.where(heads_c[None, :] == heads_d[:, None], 0.0, NEG).astype(np.float32)

    consts = {
        "wk_dr": wk_dr, "wk8": wk8,
        "wa_dr": wa_dr, "wa8": wa8,
        "wq_dr": wq_dr, "wq8": wq8,
        "wvT": np.ascontiguousarray(wv[:, :, 0, 0].T).astype(BF16),
        "woutT": np.ascontiguousarray(wout[:, :, 0, 0].T).astype(BF16),
        "ta": np.repeat(temp_a.ravel(), 16).reshape(C, 1).astype(np.float32),
        "tv": np.repeat(temp_v.ravel(), 8).reshape(64, 1).astype(np.float32),
        "mask1": mask1,
        "mask2": mask2,
    }
    return [dict(consts, xpad=xp[b], illu=il[b]) for b in range(B)]


LAST_RESULTS = None


def kernel(x, illu_feat, wq, wk, wa_dw, wa_pw, wv, wout, temp_a, temp_v):
    global LAST_RESULTS
    reps = int(_CACHE.get("reps", 1))
    loop_n = _CACHE.get("loop_n")
    key = f"nc{reps}_{loop_n}"
    if key not in _CACHE:
        _CACHE[key] = _build(reps, loop_n=loop_n)
    nc = _CACHE[key]

    in_maps = _prep_inputs(
        np.asarray(x, np.float32), np.asarray(illu_feat, np.float32),
        np.asarray(wq, np.float32), np.asarray(wk, np.float32),
        np.asarray(wa_dw, np.float32), np.asarray(wa_pw, np.float32),
        np.asarray(wv, np.float32), np.asarray(wout, np.float32),
        np.asarray(temp_a, np.float32), np.asarray(temp_v, np.float32),
    )

    from concourse.bass_utils import run_bass_kernel_spmd

    res = run_bass_kernel_spmd(nc, in_maps, core_ids=list(range(B)))
    LAST_RESULTS = res
    out = np.stack([np.asarray(res.results[b]["out"], np.float32) for b in range(B)])
    out = out.reshape(B, C, H, W)
    inv = np.empty_like(out)
    inv[:, :, :, WLIST] = out
    return inv
